# revision 5
# baseline (speedup 1.0000x reference)
"""Distributed causal attention kernel for one TRN2 chip (8 NeuronCores).

Problem: B=4, T=2048, E=1024 single-head causal attention with QKV
projections (torch-Linear convention: y = x @ W.T + b).

Sharding (data-parallel, zero collectives):
  8 cores = 4 batches x 2 query-groups. Core c handles batch b=c//2 and
  the 8 query tiles {2s+par : s=0..7} (par=c%2) of 128 rows each.
  Interleaving query tiles by parity balances the causal work: slot s on
  either core attends exactly E_s = 256*(s+1) keys (the even-parity core
  pads one fully-masked key tile).  Each core duplicates its batch's K/V
  projection -- cheaper than exchanging 8MB over the ~35GB/s intra-chip
  collective fabric.

All matmuls run in bf16 (1 cycle/row on the PE vs 4 for f32; end-to-end
rel-err ~3e-3).  Scores are computed TRANSPOSED (scoresT[t2, q]) so the
attention-probability matrix is already in lhsT layout for the AV matmul
-- no PE transposes.  Softmax is max-free (scores are provably small:
|qk/32| < ~5) and the denominator l = sum(probs) comes from an extra N=1
ones-column matmul; normalization is a per-partition scale on the way
out of PSUM.
"""

import math
import os

import numpy as np
import ml_dtypes

import concourse.bass as bass
import concourse.tile as tile
from concourse import bacc, mybir
from concourse.bass_utils import run_bass_kernel_spmd

P = 128          # partition dim / tile unit
E = 1024         # n_embd
T = 2048         # sequence length
B = 4            # batch
OC = E // P      # 8 e/o chunks
S = 8            # query slots (128-row q tiles) per core
TC = T // P      # 16 key chunks
NEG = -1e9
BF = mybir.dt.bfloat16
F32 = mybir.dt.float32
SCALE = 1.0 / math.sqrt(E)

# per key-chunk j: q columns [q0(j), 1024) participate
def _q0(j):
    return P * (j // 2)

_NQ = [S * P - _q0(j) for j in range(TC)]
_OFF = np.concatenate([[0], np.cumsum(_NQ)]).tolist()  # probsT column offsets
_PROBS_COLS = int(_OFF[-1])  # 9216


def _subchunks(n, step=512):
    out = []
    c = 0
    while c < n:
        out.append((c, min(step, n - c)))
        c += step
    return out


def build_nc():
    nc = bacc.Bacc("TRN2", target_bir_lowering=False, debug=False, num_devices=8)

    qT = nc.declare_dram_parameter("qT", [E, S * P], BF, isOutput=False)
    kT = nc.declare_dram_parameter("kT", [E, T], BF, isOutput=False)
    vT = nc.declare_dram_parameter("vT", [E, T], BF, isOutput=False)
    wqT = nc.declare_dram_parameter("wqT", [E, E], BF, isOutput=False)
    wkT = nc.declare_dram_parameter("wkT", [E, E], BF, isOutput=False)
    wvT = nc.declare_dram_parameter("wvT", [E, E], BF, isOutput=False)
    bqr = nc.declare_dram_parameter("bqr", [P, OC], F32, isOutput=False)
    bkr = nc.declare_dram_parameter("bkr", [P, OC], F32, isOutput=False)
    bvr = nc.declare_dram_parameter("bvr", [1, E], F32, isOutput=False)
    maskT = nc.declare_dram_parameter("maskT", [P, 2 * P], F32, isOutput=False)
    out_ext = nc.declare_dram_parameter("out", [S * P, E], F32, isOutput=True)

    with tile.TileContext(nc) as tc:
        with (
            tc.tile_pool(name="singles", bufs=1) as singles,
            tc.tile_pool(name="stream", bufs=2) as stream,
            tc.tile_pool(name="outp", bufs=2) as outp,
            tc.tile_pool(name="mmps", bufs=2, space="PSUM") as mmps,
            tc.tile_pool(name="avps", bufs=1, space="PSUM") as avps,
        ):
            dma = nc.sync

            # ---------- resident tiles ----------
            wq_sb = singles.tile([P, OC, E], BF)
            wk_sb = singles.tile([P, OC, E], BF)
            wv_sb = singles.tile([P, OC, E], BF)
            dma.dma_start(out=wk_sb, in_=wkT.ap().rearrange("(c p) o -> p c o", p=P))
            dma.dma_start(out=wq_sb, in_=wqT.ap().rearrange("(c p) o -> p c o", p=P))
            dma.dma_start(out=wv_sb, in_=wvT.ap().rearrange("(c p) o -> p c o", p=P))

            bq_sb = singles.tile([P, OC], F32)
            bk_sb = singles.tile([P, OC], F32)
            bv_sb = singles.tile([P, E], F32)  # bv broadcast across partitions
            mask_sb = singles.tile([P, 2 * P], F32)
            dma.dma_start(out=bq_sb, in_=bqr.ap())
            dma.dma_start(out=bk_sb, in_=bkr.ap())
            bv_ap = bvr.ap()
            dma.dma_start(
                out=bv_sb,
                in_=bass.AP(
                    tensor=bv_ap.tensor, offset=bv_ap.offset, ap=[[0, P], [1, E]]
                ),
            )
            dma.dma_start(out=mask_sb, in_=maskT.ap())

            ones_sb = singles.tile([P, P], BF)
            nc.vector.memset(ones_sb, 1.0)

            qpT = singles.tile([P, OC, S * P], BF)   # [p, o-chunk, q]
            kpT = singles.tile([P, OC, T], BF)       # [p, o-chunk, t2]
            vp = singles.tile([P, TC, E], BF)        # [p, t2-chunk, e]
            probsT = singles.tile([P, _PROBS_COLS], BF)
            recip_sb = singles.tile([P, S], F32)

            # ---------- K projection: kpT[o, t2] ----------
            for kq in range(T // 512):
                kraw = stream.tile([P, OC, 512], BF, tag="kraw")
                dma.dma_start(
                    out=kraw,
                    in_=kT.ap()[:, 512 * kq : 512 * (kq + 1)].rearrange(
                        "(c p) t -> p c t", p=P
                    ),
                )
                for o in range(OC):
                    acc = mmps.tile([P, 512], F32, tag="mm")
                    for e in range(OC):
                        nc.tensor.matmul(
                            acc,
                            lhsT=wk_sb[:, e, o * P : (o + 1) * P],
                            rhs=kraw[:, e, :],
                            start=(e == 0),
                            stop=(e == OC - 1),
                        )
                    nc.vector.tensor_scalar(
                        out=kpT[:, o, 512 * kq : 512 * (kq + 1)],
                        in0=acc,
                        scalar1=bk_sb[:, o : o + 1],
                        scalar2=None,
                        op0=mybir.AluOpType.add,
                    )

            # ---------- Q projection: qpT[o, q] ----------
            for qq in range(S * P // 512):
                qraw = stream.tile([P, OC, 512], BF, tag="qraw")
                dma.dma_start(
                    out=qraw,
                    in_=qT.ap()[:, 512 * qq : 512 * (qq + 1)].rearrange(
                        "(c p) t -> p c t", p=P
                    ),
                )
                for o in range(OC):
                    acc = mmps.tile([P, 512], F32, tag="mm")
                    for e in range(OC):
                        nc.tensor.matmul(
                            acc,
                            lhsT=wq_sb[:, e, o * P : (o + 1) * P],
                            rhs=qraw[:, e, :],
                            start=(e == 0),
                            stop=(e == OC - 1),
                        )
                    nc.vector.tensor_scalar(
                        out=qpT[:, o, 512 * qq : 512 * (qq + 1)],
                        in0=acc,
                        scalar1=bq_sb[:, o : o + 1],
                        scalar2=None,
                        op0=mybir.AluOpType.add,
                    )

            # ---------- interleaved V-projection / scores / AV ----------
            for j in range(TC):
                # V projection for t2-chunk j (vp[j] = vT[:,j].T @ wv + bv)
                if j % 4 == 0:
                    vraw = stream.tile([P, OC, 512], BF, tag="vraw")
                    dma.dma_start(
                        out=vraw,
                        in_=vT.ap()[:, 512 * (j // 4) : 512 * (j // 4 + 1)].rearrange(
                            "(c p) t -> p c t", p=P
                        ),
                    )
                jl = j % 4
                for eh in range(2):
                    acc = mmps.tile([P, 512], F32, tag="mm")
                    for e in range(OC):
                        nc.tensor.matmul(
                            acc,
                            lhsT=vraw[:, e, jl * P : (jl + 1) * P],
                            rhs=wv_sb[:, e, 512 * eh : 512 * (eh + 1)],
                            start=(e == 0),
                            stop=(e == OC - 1),
                        )
                    nc.vector.tensor_add(
                        out=vp[:, j, 512 * eh : 512 * (eh + 1)],
                        in0=acc,
                        in1=bv_sb[:, 512 * eh : 512 * (eh + 1)],
                    )

                # scoresT chunk j: [t2=128, q=Nq]
                q0 = _q0(j)
                nq = _NQ[j]
                st = mmps.tile([P, nq], F32, tag="mm")
                for o in range(OC):
                    for c0, cw in _subchunks(nq):
                        nc.tensor.matmul(
                            st[:, c0 : c0 + cw],
                            lhsT=kpT[:, o, j * P : (j + 1) * P],
                            rhs=qpT[:, o, q0 + c0 : q0 + c0 + cw],
                            start=(o == 0),
                            stop=(o == OC - 1),
                        )
                # causal mask on the first 128 q columns (slot j//2)
                nc.vector.tensor_add(
                    out=st[:, 0:P],
                    in0=st[:, 0:P],
                    in1=mask_sb[:, (j % 2) * P : (j % 2 + 1) * P],
                )
                # probsT = exp(scoresT / sqrt(E))
                nc.scalar.activation(
                    out=probsT[:, _OFF[j] : _OFF[j] + nq],
                    in_=st,
                    func=mybir.ActivationFunctionType.Exp,
                    scale=SCALE,
                )

                # AV for slot s = (j-1)//2 once its last chunk (j=2s+1) is done
                if j % 2 == 1:
                    s = j // 2
                    nchunks = j + 1
                    av = avps.tile([P, 1536], F32, tag="av")
                    for jj in range(nchunks):
                        lhsT = probsT[
                            :,
                            _OFF[jj]
                            + (s - jj // 2) * P : _OFF[jj]
                            + (s - jj // 2) * P
                            + P,
                        ]
                        st_f = jj == 0
                        sp_f = jj == nchunks - 1
                        for eh in range(2):
                            nc.tensor.matmul(
                                av[:, 512 * eh : 512 * (eh + 1)],
                                lhsT=lhsT,
                                rhs=vp[:, jj, 512 * eh : 512 * (eh + 1)],
                                start=st_f,
                                stop=sp_f,
                            )
                        nc.tensor.matmul(
                            av[:, 1024:1025],
                            lhsT=lhsT,
                            rhs=ones_sb[:, 0:1],
                            start=st_f,
                            stop=sp_f,
                        )
                    nc.vector.reciprocal(
                        out=recip_sb[:, s : s + 1], in_=av[:, 1024:1025]
                    )
                    osb = outp.tile([P, E], F32, tag="osb")
                    nc.scalar.mul(out=osb, in_=av[:, 0:1024], mul=recip_sb[:, s : s + 1])
                    dma.dma_start(out=out_ext.ap()[P * s : P * (s + 1), :], in_=osb)

    nc.finalize()
    return nc


_NC_CACHE = {}


def _get_nc():
    if "nc" not in _NC_CACHE:
        _NC_CACHE["nc"] = build_nc()
    return _NC_CACHE["nc"]


def _bf16(x):
    return np.asarray(x, np.float32).astype(ml_dtypes.bfloat16)


def make_in_maps(q, k, v, wq_w, wq_b, wk_w, wk_b, wv_w, wv_b):
    """Host-side sharding: returns list of 8 per-core input dicts."""
    q = np.asarray(q, np.float32)
    k = np.asarray(k, np.float32)
    v = np.asarray(v, np.float32)
    wqT = _bf16(np.asarray(wq_w).T)
    wkT = _bf16(np.asarray(wk_w).T)
    wvT = _bf16(np.asarray(wv_w).T)
    bqr = np.ascontiguousarray(
        np.asarray(wq_b, np.float32).reshape(OC, P).T
    )
    bkr = np.ascontiguousarray(
        np.asarray(wk_b, np.float32).reshape(OC, P).T
    )
    bvr = np.asarray(wv_b, np.float32).reshape(1, E)

    r = np.arange(P)
    tril = np.where(r[:, None] <= r[None, :], 0.0, NEG).astype(np.float32)
    mask_even = np.concatenate([tril, np.full((P, P), NEG, np.float32)], axis=1)
    mask_odd = np.concatenate([np.zeros((P, P), np.float32), tril], axis=1)

    in_maps = []
    for c in range(8):
        b, par = c // 2, c % 2
        rows = np.concatenate(
            [np.arange(P * (2 * s + par), P * (2 * s + par) + P) for s in range(S)]
        )
        in_maps.append(
            {
                "qT": np.ascontiguousarray(_bf16(q[b][rows]).T),
                "kT": np.ascontiguousarray(_bf16(k[b]).T),
                "vT": np.ascontiguousarray(_bf16(v[b]).T),
                "wqT": wqT,
                "wkT": wkT,
                "wvT": wvT,
                "bqr": bqr,
                "bkr": bkr,
                "bvr": bvr,
                "maskT": mask_even if par == 0 else mask_odd,
            }
        )
    return in_maps


def assemble_out(per_core_outs):
    """Inverse of the query sharding: returns [B, T, E] f32."""
    out = np.empty((B, T, E), np.float32)
    for c in range(8):
        b, par = c // 2, c % 2
        o = np.asarray(per_core_outs[c])
        for s in range(S):
            out[b, P * (2 * s + par) : P * (2 * s + par) + P, :] = o[
                P * s : P * (s + 1), :
            ]
    return out


def _kernel_np_fallback(q, k, v, wq_w, wq_b, wk_w, wk_b, wv_w, wv_b, causal):
    """Numpy reference path (used only for the causal=0 edge case)."""
    q = np.asarray(q, np.float32)
    out = np.empty_like(q)
    for b in range(q.shape[0]):
        qp = q[b] @ np.asarray(wq_w, np.float32).T + np.asarray(wq_b, np.float32)
        kp = np.asarray(k[b], np.float32) @ np.asarray(wk_w, np.float32).T + np.asarray(
            wk_b, np.float32
        )
        vp = np.asarray(v[b], np.float32) @ np.asarray(wv_w, np.float32).T + np.asarray(
            wv_b, np.float32
        )
        s = (qp @ kp.T) * SCALE
        if causal:
            t = s.shape[0]
            s = np.where(np.tril(np.ones((t, t), bool)), s, -np.inf)
        s -= s.max(-1, keepdims=True)
        p = np.exp(s)
        out[b] = (p @ vp) / p.sum(-1, keepdims=True)
    return out


def kernel(q, k, v, wq_w, wq_b, wk_w, wk_b, wv_w, wv_b, causal, **run_kwargs):
    if not int(causal):
        return _kernel_np_fallback(
            q, k, v, wq_w, wq_b, wk_w, wk_b, wv_w, wv_b, causal
        )
    nc = _get_nc()
    in_maps = make_in_maps(q, k, v, wq_w, wq_b, wk_w, wk_b, wv_w, wv_b)
    res = run_bass_kernel_spmd(nc, in_maps, core_ids=list(range(8)), **run_kwargs)
    out = assemble_out([r["out"] for r in res.results])
    if run_kwargs:
        kernel.last_results = res
    return out


# revision 9
# speedup vs baseline: 1.0500x; 1.0500x over previous
"""Distributed causal attention kernel for one TRN2 chip (8 NeuronCores).

Problem: B=4, T=2048, E=1024 single-head causal attention with QKV
projections (torch-Linear convention: y = x @ W.T + b).

Sharding (data-parallel, zero collectives):
  8 cores = 4 batches x 2 query-groups. Core c handles batch b=c//2 and
  the 8 query tiles {2s+par : s=0..7} (par=c%2) of 128 rows each.
  Interleaving query tiles by parity balances the causal work: slot s on
  either core attends exactly E_s = 256*(s+1) keys (the even-parity core
  pads one fully-masked 128x128 block per slot).  Each core duplicates
  its batch's K/V projection -- cheaper than exchanging 8MB over the
  ~35GB/s intra-chip collective fabric.

Precision strategy:
  - All matmuls run in bf16 (1 cycle/row on the PE vs 4 for f32);
    end-to-end rel err ~3.4e-3.  fp8e4m3+DoubleRow was tried for the
    Q/K path and rejected: 3-bit mantissa puts ~6.5% noise on the
    softmax logits, which lands ~1:1 as ~2.3% output error.
  - Scores are computed TRANSPOSED (scoresT[t2, q]) so the probability
    matrix is already in lhsT layout for AV -- no PE transposes.
    Softmax is max-free (|logits| < ~5 by construction) and the
    denominator comes from an extra N=1 ones-column matmul; the final
    normalization is a per-partition scale on the PSUM->SBUF eviction.
"""

import math

import numpy as np
import ml_dtypes

import concourse.bass as bass
import concourse.tile as tile
from concourse import bacc, mybir
from concourse.bass_utils import run_bass_kernel_spmd

P = 128          # partition dim / tile unit
E = 1024         # n_embd
T = 2048         # sequence length
B = 4            # batch
OC = E // P      # 8 e/o chunks
S = 8            # query slots (128-row q tiles) per core
TC = T // P      # 16 key chunks
NEG = -1e9
BF = mybir.dt.bfloat16
F8 = mybir.dt.float8e4
F32 = mybir.dt.float32
DR = mybir.MatmulPerfMode.DoubleRow
SCALE = 1.0 / math.sqrt(E)
W_SCALE = 64.0   # fp8 weight pre-scale (host) / rescale (eviction)

# per key-chunk j: q columns [q0(j), 1024) participate
def _q0(j):
    return P * (j // 2)

_NQ = [S * P - _q0(j) for j in range(TC)]
_OFF = np.concatenate([[0], np.cumsum(_NQ)]).tolist()  # probsT column offsets
_PROBS_COLS = int(_OFF[-1])  # 9216


def _subchunks(n, step=512):
    out = []
    c = 0
    while c < n:
        out.append((c, min(step, n - c)))
        c += step
    return out


def build_nc():
    nc = bacc.Bacc("TRN2", target_bir_lowering=False, debug=False, num_devices=8)

    qT = nc.declare_dram_parameter("qT", [E, S * P], BF, isOutput=False)
    kT = nc.declare_dram_parameter("kT", [E, T], BF, isOutput=False)
    vT = nc.declare_dram_parameter("vT", [E, T], BF, isOutput=False)
    wqT = nc.declare_dram_parameter("wqT", [E, E], BF, isOutput=False)
    wkT = nc.declare_dram_parameter("wkT", [E, E], BF, isOutput=False)
    wvT = nc.declare_dram_parameter("wvT", [E, E], BF, isOutput=False)
    bqr = nc.declare_dram_parameter("bqr", [P, OC], F32, isOutput=False)
    bkr = nc.declare_dram_parameter("bkr", [P, OC], F32, isOutput=False)
    bvr = nc.declare_dram_parameter("bvr", [1, E], F32, isOutput=False)
    maskT = nc.declare_dram_parameter("maskT", [P, 2 * P], F32, isOutput=False)
    out_ext = nc.declare_dram_parameter("out", [S * P, E], F32, isOutput=True)

    with tile.TileContext(nc) as tc:
        with (
            tc.tile_pool(name="singles", bufs=1) as singles,
            tc.tile_pool(name="stream", bufs=2) as stream,
            tc.tile_pool(name="outp", bufs=2) as outp,
            tc.tile_pool(name="mmps", bufs=2, space="PSUM") as mmps,
            tc.tile_pool(name="avps", bufs=1, space="PSUM") as avps,
        ):
            # two independent DMA issue queues: critical-path stream tiles on
            # sync, bulk weights/constants on gpsimd
            dma = nc.sync
            dma2 = nc.gpsimd

            # ---------- resident tiles (wk first: kp is the first compute) ----------
            wq_sb = singles.tile([P, OC, E], BF)
            wk_sb = singles.tile([P, OC, E], BF)
            wv_sb = singles.tile([P, OC, E], BF)
            dma2.dma_start(out=wk_sb, in_=wkT.ap().rearrange("(c p) o -> p c o", p=P))
            dma2.dma_start(out=wq_sb, in_=wqT.ap().rearrange("(c p) o -> p c o", p=P))

            bq_sb = singles.tile([P, OC], F32)
            bk_sb = singles.tile([P, OC], F32)
            bv_sb = singles.tile([P, E], F32)  # bv broadcast across partitions
            mask_sb = singles.tile([P, 2 * P], F32)
            dma2.dma_start(out=bk_sb, in_=bkr.ap())
            dma2.dma_start(out=bq_sb, in_=bqr.ap())
            dma2.dma_start(out=mask_sb, in_=maskT.ap())

            ones_sb = singles.tile([P, P], BF)
            nc.vector.memset(ones_sb, 1.0)

            qpT = singles.tile([P, OC, S * P], BF)   # [p, o-chunk, q]
            kpT = singles.tile([P, OC, T], BF)       # [p, o-chunk, t2]
            vp = singles.tile([P, TC, E], BF)        # [p, t2-chunk, e]
            probsT = singles.tile([P, _PROBS_COLS], BF)
            recip_sb = singles.tile([P, S], F32)

            # ---------- K projection: kpT[o, t2] ----------
            for kq in range(T // 512):
                kraw = stream.tile([P, OC, 512], BF, tag="kraw")
                dma.dma_start(
                    out=kraw,
                    in_=kT.ap()[:, 512 * kq : 512 * (kq + 1)].rearrange(
                        "(c p) t -> p c t", p=P
                    ),
                )
                for o in range(OC):
                    acc = mmps.tile([P, 512], F32, tag="mm")
                    for e in range(OC):
                        nc.tensor.matmul(
                            acc,
                            lhsT=wk_sb[:, e, o * P : (o + 1) * P],
                            rhs=kraw[:, e, :],
                            start=(e == 0),
                            stop=(e == OC - 1),
                        )
                    nc.vector.tensor_scalar(
                        out=kpT[:, o, 512 * kq : 512 * (kq + 1)],
                        in0=acc,
                        scalar1=bk_sb[:, o : o + 1],
                        scalar2=None,
                        op0=mybir.AluOpType.add,
                    )

            # ---------- Q projection: qpT[o, q] ----------
            for qq in range(S * P // 512):
                qraw = stream.tile([P, OC, 512], BF, tag="qraw")
                dma.dma_start(
                    out=qraw,
                    in_=qT.ap()[:, 512 * qq : 512 * (qq + 1)].rearrange(
                        "(c p) t -> p c t", p=P
                    ),
                )
                for o in range(OC):
                    acc = mmps.tile([P, 512], F32, tag="mm")
                    for e in range(OC):
                        nc.tensor.matmul(
                            acc,
                            lhsT=wq_sb[:, e, o * P : (o + 1) * P],
                            rhs=qraw[:, e, :],
                            start=(e == 0),
                            stop=(e == OC - 1),
                        )
                    nc.vector.tensor_scalar(
                        out=qpT[:, o, 512 * qq : 512 * (qq + 1)],
                        in0=acc,
                        scalar1=bq_sb[:, o : o + 1],
                        scalar2=None,
                        op0=mybir.AluOpType.add,
                    )

            # wv/bv arrive while projections run
            dma2.dma_start(out=wv_sb, in_=wvT.ap().rearrange("(c p) o -> p c o", p=P))
            bv_ap = bvr.ap()
            dma2.dma_start(
                out=bv_sb,
                in_=bass.AP(
                    tensor=bv_ap.tensor, offset=bv_ap.offset, ap=[[0, P], [1, E]]
                ),
            )

            # ---------- interleaved V-projection / scores / AV ----------
            for j in range(TC):
                # V projection for t2-chunk j (vp[j] = vT[:,j].T @ wv + bv), bf16
                if j % 4 == 0:
                    vraw = stream.tile([P, OC, 512], BF, tag="vraw")
                    dma2.dma_start(
                        out=vraw,
                        in_=vT.ap()[:, 512 * (j // 4) : 512 * (j // 4 + 1)].rearrange(
                            "(c p) t -> p c t", p=P
                        ),
                    )
                jl = j % 4
                for eh in range(2):
                    acc = mmps.tile([P, 512], F32, tag="mm")
                    for e in range(OC):
                        nc.tensor.matmul(
                            acc,
                            lhsT=vraw[:, e, jl * P : (jl + 1) * P],
                            rhs=wv_sb[:, e, 512 * eh : 512 * (eh + 1)],
                            start=(e == 0),
                            stop=(e == OC - 1),
                        )
                    nc.vector.tensor_add(
                        out=vp[:, j, 512 * eh : 512 * (eh + 1)],
                        in0=acc,
                        in1=bv_sb[:, 512 * eh : 512 * (eh + 1)],
                    )

                # scoresT chunk j: [t2=128, q=Nq]
                q0 = _q0(j)
                nq = _NQ[j]
                st = mmps.tile([P, nq], F32, tag="mm")
                for o in range(OC):
                    for c0, cw in _subchunks(nq):
                        nc.tensor.matmul(
                            st[:, c0 : c0 + cw],
                            lhsT=kpT[:, o, j * P : (j + 1) * P],
                            rhs=qpT[:, o, q0 + c0 : q0 + c0 + cw],
                            start=(o == 0),
                            stop=(o == OC - 1),
                        )
                # causal mask on the first 128 q columns (slot j//2)
                nc.vector.tensor_add(
                    out=st[:, 0:P],
                    in0=st[:, 0:P],
                    in1=mask_sb[:, (j % 2) * P : (j % 2 + 1) * P],
                )
                # probsT = exp(scoresT / sqrt(E))
                nc.scalar.activation(
                    out=probsT[:, _OFF[j] : _OFF[j] + nq],
                    in_=st,
                    func=mybir.ActivationFunctionType.Exp,
                    scale=SCALE,
                )

                # AV for slot s = (j-1)//2 once its last chunk (j=2s+1) is done
                if j % 2 == 1:
                    s = j // 2
                    nchunks = j + 1
                    av = avps.tile([P, 1536], F32, tag="av")
                    for jj in range(nchunks):
                        lhsT = probsT[
                            :,
                            _OFF[jj]
                            + (s - jj // 2) * P : _OFF[jj]
                            + (s - jj // 2) * P
                            + P,
                        ]
                        st_f = jj == 0
                        sp_f = jj == nchunks - 1
                        for eh in range(2):
                            nc.tensor.matmul(
                                av[:, 512 * eh : 512 * (eh + 1)],
                                lhsT=lhsT,
                                rhs=vp[:, jj, 512 * eh : 512 * (eh + 1)],
                                start=st_f,
                                stop=sp_f,
                            )
                        nc.tensor.matmul(
                            av[:, 1024:1025],
                            lhsT=lhsT,
                            rhs=ones_sb[:, 0:1],
                            start=st_f,
                            stop=sp_f,
                        )
                    nc.vector.reciprocal(
                        out=recip_sb[:, s : s + 1], in_=av[:, 1024:1025]
                    )
                    osb = outp.tile([P, E], F32, tag="osb")
                    nc.scalar.mul(out=osb, in_=av[:, 0:1024], mul=recip_sb[:, s : s + 1])
                    dma.dma_start(out=out_ext.ap()[P * s : P * (s + 1), :], in_=osb)

    nc.finalize()
    return nc


_NC_CACHE = {}


def _get_nc():
    if "nc" not in _NC_CACHE:
        _NC_CACHE["nc"] = build_nc()
    return _NC_CACHE["nc"]


def _bf16(x):
    return np.asarray(x, np.float32).astype(ml_dtypes.bfloat16)


def _f8(x):
    return np.clip(np.asarray(x, np.float32), -240.0, 240.0).astype(
        ml_dtypes.float8_e4m3
    )


def make_in_maps(q, k, v, wq_w, wq_b, wk_w, wk_b, wv_w, wv_b):
    """Host-side sharding: returns list of 8 per-core input dicts."""
    q = np.asarray(q, np.float32)
    k = np.asarray(k, np.float32)
    v = np.asarray(v, np.float32)
    wqT = _bf16(np.asarray(wq_w).T)
    wkT = _bf16(np.asarray(wk_w).T)
    wvT = _bf16(np.asarray(wv_w).T)
    bqr = np.ascontiguousarray(np.asarray(wq_b, np.float32).reshape(OC, P).T)
    bkr = np.ascontiguousarray(np.asarray(wk_b, np.float32).reshape(OC, P).T)
    bvr = np.asarray(wv_b, np.float32).reshape(1, E)

    r = np.arange(P)
    tril = np.where(r[:, None] <= r[None, :], 0.0, NEG).astype(np.float32)
    mask_even = np.concatenate([tril, np.full((P, P), NEG, np.float32)], axis=1)
    mask_odd = np.concatenate([np.zeros((P, P), np.float32), tril], axis=1)

    in_maps = []
    for c in range(8):
        b, par = c // 2, c % 2
        rows = np.concatenate(
            [np.arange(P * (2 * s + par), P * (2 * s + par) + P) for s in range(S)]
        )
        in_maps.append(
            {
                "qT": np.ascontiguousarray(_bf16(q[b][rows]).T),
                "kT": np.ascontiguousarray(_bf16(k[b]).T),
                "vT": np.ascontiguousarray(_bf16(v[b]).T),
                "wqT": wqT,
                "wkT": wkT,
                "wvT": wvT,
                "bqr": bqr,
                "bkr": bkr,
                "bvr": bvr,
                "maskT": mask_even if par == 0 else mask_odd,
            }
        )
    return in_maps


def assemble_out(per_core_outs):
    """Inverse of the query sharding: returns [B, T, E] f32."""
    out = np.empty((B, T, E), np.float32)
    for c in range(8):
        b, par = c // 2, c % 2
        o = np.asarray(per_core_outs[c])
        for s in range(S):
            out[b, P * (2 * s + par) : P * (2 * s + par) + P, :] = o[
                P * s : P * (s + 1), :
            ]
    return out


def _kernel_np_fallback(q, k, v, wq_w, wq_b, wk_w, wk_b, wv_w, wv_b, causal):
    """Numpy reference path (used only for the causal=0 edge case)."""
    q = np.asarray(q, np.float32)
    out = np.empty_like(q)
    for b in range(q.shape[0]):
        qp = q[b] @ np.asarray(wq_w, np.float32).T + np.asarray(wq_b, np.float32)
        kp = np.asarray(k[b], np.float32) @ np.asarray(wk_w, np.float32).T + np.asarray(
            wk_b, np.float32
        )
        vp = np.asarray(v[b], np.float32) @ np.asarray(wv_w, np.float32).T + np.asarray(
            wv_b, np.float32
        )
        s = (qp @ kp.T) * SCALE
        if causal:
            t = s.shape[0]
            s = np.where(np.tril(np.ones((t, t), bool)), s, -np.inf)
        s -= s.max(-1, keepdims=True)
        p = np.exp(s)
        out[b] = (p @ vp) / p.sum(-1, keepdims=True)
    return out


def kernel(q, k, v, wq_w, wq_b, wk_w, wk_b, wv_w, wv_b, causal, **run_kwargs):
    if not int(causal):
        return _kernel_np_fallback(
            q, k, v, wq_w, wq_b, wk_w, wk_b, wv_w, wv_b, causal
        )
    nc = _get_nc()
    in_maps = make_in_maps(q, k, v, wq_w, wq_b, wk_w, wk_b, wv_w, wv_b)
    res = run_bass_kernel_spmd(nc, in_maps, core_ids=list(range(8)), **run_kwargs)
    out = assemble_out([r["out"] for r in res.results])
    if run_kwargs:
        kernel.last_results = res
    return out
